# revision 2
# baseline (speedup 1.0000x reference)
"""Grouped GEMM (MoE routing) kernel for 8 Trainium2 NeuronCores.

out[off_g : off_g + size_g] = A[off_g : off_g + size_g] @ B[g]   for g in 0..63
A: [524288, 256] f32, B: [64, 256, 256] f32, groups are contiguous row ranges.

Strategy (hardcoded, from the sharding hint "expert-parallel"):
  - Every core runs an IDENTICAL static schedule of T rows; the per-core
    group assignment lives in the DATA (host-packed AT / BW tensors).
  - Packing: constrained LPT assigns 8 groups per core (balanced totals),
    then a local-search swap pass minimizes T.  Slot structure = 8 "bulk"
    slots (budget = min of the i-th largest group over cores) + ~7
    "cleanup" slots (sorted leftover tails, budget = max over cores).
    T ~ 66240 rows (1.1% padding) vs 69632 for one-group-per-slot octiles.
  - bf16 operands + bf16 output (accumulation stays f32 in PSUM).
  - Host packs each core's chunks back to back, pre-transposed to
    AT [256, T] bf16; device computes the TRANSPOSED output OUTT [256, T].
  - Uniform ~4K-row blocks tapered at both ends.  Per segment and output
    half h (128 of 256 N cols): stream <=512-row matmul spans (one PSUM
    bank each; h=0 rotates banks 0-3, h=1 banks 4-7), then cast-copy
    PSUM->SBUF on vector (h=0) / scalar (h=1) so both drain concurrently.
  - DMA queues: loads a0->sync, a1->scalar (HWDGE); steady stores
    h0->gpsimd (SWDGE), h1->sync.  Weights are issued AFTER the first two
    blocks' loads, split so schedule-order slots arrive first.  The last
    TAILROT blocks rotate stores over all 3 queues (loads are done by
    then, so the scalar ring is free) to speed the drain.
"""

import os
import numpy as np

NCORES = 8
K = 256
N = 256
SPAN = 512       # rows per PSUM bank (512 f32 = 2KB = one bank)
WROWS = int(os.environ.get("BASS_GG_W", "4096"))    # rows per A/out block
GROUPN = int(os.environ.get("BASS_GG_GROUP", "4"))  # spans per PSUM bank group
ABUFS = int(os.environ.get("BASS_GG_ABUFS", "6"))
OBUFS = int(os.environ.get("BASS_GG_OBUFS", "6"))
LOOKAHEAD = int(os.environ.get("BASS_GG_LOOKAHEAD", "5"))
TAILROT = int(os.environ.get("BASS_GG_TAILROT", "6"))  # blocks with rotated stores
LEAD = tuple(int(x) for x in os.environ.get("BASS_GG_LEAD", "1024,1024,2048").split(",") if x)
TAIL = tuple(int(x) for x in os.environ.get("BASS_GG_TAIL", "2048,1024,512,512").split(",") if x)
CLEAN_AT = float(os.environ.get("BASS_GG_CLEANAT", "0.55"))  # cleanup position

LAST_EXEC_NS = None  # set when BASS_GG_TRACE=1
LAST_EXEC_LIST = []

_prog_cache = {}


def _schedule(sizes):
    """Pack groups into an identical-per-core slot schedule.

    Returns (r_list, plan) where r_list[i] is slot i's row budget and
    plan[c][i] = (group_id, group_row_start, nrows) for core c, slot i.
    Groups are padded to 64-row multiples (the pad rows are zeros).
    """
    import random

    sizes = np.asarray(sizes, dtype=np.int64)
    g = sizes.shape[0]
    pad_groups = (-g) % NCORES
    if pad_groups:
        sizes = np.concatenate([sizes, np.zeros(pad_groups, np.int64)])
        g += pad_groups
    s64 = ((sizes + 63) // 64 * 64).astype(np.int64)
    per = g // NCORES

    # constrained LPT: exactly `per` groups per core, balanced row totals
    order = np.argsort(-s64, kind="stable")
    tot = np.zeros(NCORES, np.int64)
    cnt = np.zeros(NCORES, np.int64)
    assign = [[] for _ in range(NCORES)]
    for gid in order:
        elig = [c for c in range(NCORES) if cnt[c] < per]
        c = min(elig, key=lambda c: tot[c])
        tot[c] += s64[gid]
        cnt[c] += 1
        assign[c].append(int(gid))

    def stats(asg):
        S = np.array(
            [sorted((s64[x] for x in asg[c]), reverse=True) for c in range(NCORES)],
            dtype=np.int64,
        )
        q = S.min(axis=0)
        L = S - q[None, :]
        p = (-np.sort(-L, axis=1)).max(axis=0)
        return int(q.sum() + p[p > 0].sum()), q, p

    best, _, _ = stats(assign)
    rnd = random.Random(7)
    cur = [list(a) for a in assign]
    for _ in range(12000):
        c1, c2 = rnd.sample(range(NCORES), 2)
        i1, i2 = rnd.randrange(per), rnd.randrange(per)
        cur[c1][i1], cur[c2][i2] = cur[c2][i2], cur[c1][i1]
        t, _, _ = stats(cur)
        if t <= best:
            best = t
        else:
            cur[c1][i1], cur[c2][i2] = cur[c2][i2], cur[c1][i1]

    T, q, p = stats(cur)
    nb = len(q)
    # per-core groups sorted desc by padded size
    Gc = [sorted(cur[c], key=lambda x: -s64[x]) for c in range(NCORES)]
    # leftovers per core, sorted desc -> cleanup slots
    keepp = [int(x) for x in p if x > 0]
    ncl = len(keepp)
    bulk = []   # bulk[i] = (budget, per-core (gid, 0, nrows))
    for i in range(nb):
        entries = []
        for c in range(NCORES):
            gid = Gc[c][i]
            entries.append((gid, 0, int(q[i])))
        bulk.append((int(q[i]), entries))
    clean = []  # cleanup slot j
    lsorted = []
    for c in range(NCORES):
        lv = [(int(s64[Gc[c][i]] - q[i]), i) for i in range(nb)]
        lv.sort(key=lambda x: -x[0])
        lsorted.append(lv)
    for j in range(ncl):
        entries = []
        for c in range(NCORES):
            l, i = lsorted[c][j]
            gid = Gc[c][i]
            entries.append((gid, int(q[i]), l))
        clean.append((keepp[j], entries))

    # order slots: bulk prefix, cleanup in the middle, bulk suffix
    slots = []
    cum = 0
    inserted = False
    for bslot in bulk:
        if not inserted and cum >= CLEAN_AT * T:
            slots.extend(clean)
            inserted = True
        slots.append(bslot)
        cum += bslot[0]
    if not inserted:
        slots.extend(clean)

    r_list = [s[0] for s in slots]
    plan = [[s[1][c] for s in slots] for c in range(NCORES)]
    return r_list, plan


def _make_blocks(r_list):
    """Uniform blocks [(row0, nrows)], tapered at the schedule's ends."""
    T = int(sum(r_list))
    lead = list(LEAD)
    tail = list(TAIL)
    mid = T - sum(lead) - sum(tail)
    parts = max(1, (mid + WROWS - 1) // WROWS)
    base = (mid // parts + 63) // 64 * 64
    sizes = []
    rem = mid
    while rem > 0:
        w = min(base, rem)
        sizes.append(w)
        rem -= w
    blocks = []
    t0 = 0
    for w in lead + sizes + tail:
        blocks.append((t0, w))
        t0 += w
    assert t0 == T
    return blocks, T


def _build_program(r_list):
    import concourse.tile as tile
    from concourse import bacc, mybir

    BF16 = mybir.dt.bfloat16
    F32 = mybir.dt.float32
    R = len(r_list)

    blocks, T = _make_blocks(r_list)
    slot_start = [0]
    for r in r_list:
        slot_start.append(slot_start[-1] + int(r))

    def slot_at(row):
        for i in range(R):
            if row < slot_start[i + 1]:
                return i
        return R - 1

    nc = bacc.Bacc(
        "TRN2",
        target_bir_lowering=False,
        debug=False,
        enable_asserts=False,
        num_devices=NCORES,
    )
    AT = nc.dram_tensor("AT", [K, T], BF16, kind="ExternalInput").ap()
    BW = nc.dram_tensor("BW", [128, R, 2, 2, 128], BF16, kind="ExternalInput").ap()
    OUTT = nc.dram_tensor("OUTT", [N, T], BF16, kind="ExternalOutput").ap()

    with tile.TileContext(nc) as tc:
        with tc.tile_pool(name="bpool", bufs=1) as bpool, \
             tc.tile_pool(name="apool", bufs=ABUFS) as apool, \
             tc.tile_pool(name="opool", bufs=OBUFS) as opool, \
             tc.tile_pool(name="psum", bufs=8, space="PSUM") as pspool:
            b_sb = bpool.tile([128, R, 2, 2, 128], BF16)

            abufs = {}
            ENG = [nc.sync, nc.scalar, nc.gpsimd]

            def emit_loads(bi):
                t0, w = blocks[bi]
                a0 = apool.tile([128, WROWS], BF16, tag="a0")
                a1 = apool.tile([128, WROWS], BF16, tag="a1")
                nc.sync.dma_start(out=a0[:, :w], in_=AT[0:128, t0 : t0 + w])
                nc.scalar.dma_start(out=a1[:, :w], in_=AT[128:256, t0 : t0 + w])
                abufs[bi] = (a0, a1)

            nblk = len(blocks)

            def emit_compute(bi):
                t0, w = blocks[bi]
                a0, a1 = abufs.pop(bi)
                ob = opool.tile([128, 2, WROWS], BF16, tag="ob")

                segs = []
                off = 0
                while off < w:
                    s = slot_at(t0 + off)
                    end = min(w, slot_start[s + 1] - t0)
                    segs.append((off, end, s))
                    off = end

                for (so, se, s) in segs:
                    spans = []
                    off = so
                    while off < se:
                        spans.append((off, min(SPAN, se - off)))
                        off += spans[-1][1]
                    for c0 in range(0, len(spans), GROUPN):
                        chunk = spans[c0 : c0 + GROUPN]
                        for h in range(2):
                            pss = [
                                pspool.tile([128, SPAN], F32, name="ps")
                                for _ in chunk
                            ]
                            for j, aj in ((0, a0), (1, a1)):
                                for (off, ln), ps in zip(chunk, pss):
                                    nc.tensor.matmul(
                                        ps[:, :ln],
                                        lhsT=b_sb[:, s, j, h, :],
                                        rhs=aj[:, off : off + ln],
                                        start=(j == 0),
                                        stop=(j == 1),
                                    )
                            eng = (
                                nc.vector.tensor_copy if h == 0 else nc.scalar.copy
                            )
                            for (off, ln), ps in zip(chunk, pss):
                                eng(out=ob[:, h, off : off + ln], in_=ps[:, :ln])
                if bi >= nblk - TAILROT:
                    # drain phase: loads are finished, rotate stores over
                    # all three queues so the fabric stays busy
                    qa = ENG[(2 * bi) % 3]
                    qb = ENG[(2 * bi + 1) % 3]
                else:
                    qa, qb = nc.gpsimd, nc.sync
                for h, qq in ((0, qa), (1, qb)):
                    qq.dma_start(
                        out=OUTT[h * 128 : (h + 1) * 128, t0 : t0 + w],
                        in_=ob[:, h, :w],
                    )

            # head: first two blocks' loads go out before the weights so
            # compute can start ASAP; weights follow in schedule order
            emit_loads(0)
            emit_loads(1)
            w0 = min(2, R)
            w1 = min(6, R)
            w2 = min(11, R)
            nc.gpsimd.dma_start(out=b_sb[:, 0:w0], in_=BW[:, 0:w0])
            if w1 > w0:
                nc.gpsimd.dma_start(out=b_sb[:, w0:w1], in_=BW[:, w0:w1])
            if w2 > w1:
                nc.scalar.dma_start(out=b_sb[:, w1:w2], in_=BW[:, w1:w2])
            if R > w2:
                nc.sync.dma_start(out=b_sb[:, w2:R], in_=BW[:, w2:R])

            for bi in range(2, nblk + LOOKAHEAD):
                if bi < nblk:
                    emit_loads(bi)
                if bi >= LOOKAHEAD:
                    emit_compute(bi - LOOKAHEAD)
    nc.compile()
    return nc


def _get_program(r_key):
    key = (r_key, WROWS, GROUPN, ABUFS, OBUFS, LOOKAHEAD, TAILROT, LEAD, TAIL)
    if key not in _prog_cache:
        _prog_cache[key] = _build_program(list(r_key))
    return _prog_cache[key]


def kernel(A, B, batch_sizes, batch_offsets, batch_padded_offsets):
    global LAST_EXEC_NS
    import ml_dtypes
    from concourse.bass_utils import run_bass_kernel_spmd

    # If tracing is requested via env but the NTFF hook module is absent,
    # register a stub that reports "no hook" so bass_utils degrades to an
    # untraced run instead of crashing on the import.
    try:
        import antenv.axon_hooks  # noqa: F401
    except ImportError:
        import sys
        import types

        _m = types.ModuleType("antenv.axon_hooks")
        _m.get_axon_ntff_profile_hook = lambda: None
        sys.modules.setdefault("antenv.axon_hooks", _m)

    bf16 = ml_dtypes.bfloat16
    A = np.asarray(A, dtype=np.float32)
    B = np.asarray(B, dtype=np.float32)
    sizes = np.asarray(batch_sizes, dtype=np.int64)
    offsets = np.asarray(batch_offsets, dtype=np.int64)

    M = A.shape[0]
    G = B.shape[0]
    r_list, plan = _schedule(sizes)
    starts = np.concatenate([[0], np.cumsum(r_list)[:-1]]).astype(np.int64)
    T = int(sum(r_list))
    R = len(r_list)

    nc = _get_program(tuple(int(x) for x in r_list))

    ATfull = np.ascontiguousarray(A.astype(bf16).T)  # [K, M]
    Bbf = B.astype(bf16)  # [G, K, N]

    in_maps = []
    for c in range(NCORES):
        at = np.zeros((K, T), dtype=bf16)
        bw = np.zeros((128, R, 2, 2, 128), dtype=bf16)
        for i in range(R):
            gid, gr0, nrows = plan[c][i]
            dst = int(starts[i])
            if gid < G:
                off, sz = int(offsets[gid]), int(sizes[gid])
                lo = min(gr0, sz)
                hi = min(gr0 + nrows, sz)
                if hi > lo:
                    at[:, dst + (lo - gr0) : dst + (hi - gr0)] = ATfull[
                        :, off + lo : off + hi
                    ]
                # bw[p, i, j, h, n] = B[gid, j*128+p, h*128+n]
                bw[:, i] = Bbf[gid].reshape(2, 128, 2, 128).transpose(1, 0, 2, 3)
        in_maps.append({"AT": at, "BW": bw})

    trace = bool(int(os.environ.get("BASS_GG_TRACE", "0")))
    repeats = int(os.environ.get("BASS_GG_REPEAT", "1"))
    times = []
    for _ in range(max(1, repeats)):
        res = run_bass_kernel_spmd(
            nc, in_maps, core_ids=list(range(NCORES)), trace=trace
        )
        times.append(res.exec_time_ns)
    global LAST_EXEC_LIST
    LAST_EXEC_LIST = times
    LAST_EXEC_NS = min((t for t in times if t is not None), default=None)

    outT = np.zeros((N, M), dtype=np.float32)
    for c in range(NCORES):
        oc = res.results[c]["OUTT"]
        for i in range(R):
            gid, gr0, nrows = plan[c][i]
            src = int(starts[i])
            if gid >= G:
                continue
            off, sz = int(offsets[gid]), int(sizes[gid])
            lo = min(gr0, sz)
            hi = min(gr0 + nrows, sz)
            if hi > lo:
                outT[:, off + lo : off + hi] = oc[
                    :, src + (lo - gr0) : src + (hi - gr0)
                ]
    return outT.T
